# revision 19
# baseline (speedup 1.0000x reference)
"""Trainium2 Bass kernel for CausalDownsamplingLRU.

Per core = one batch element (8 cores, data-parallel over batch).

  h_t = lam*h_{t-1} + gamma*B x_t  (diagonal complex),  y = Re(C h) + D x,
  keep y[:, -1024:].

Mapping:
  * States are SORTED by |lam| (descending) on the host; all per-state
    tables are permuted to match.  C columns permuted too, so y is exact.
  * Carry h_1023 via a truncated W-GEMM: W[n,i] = sum_s lam^{1023-s} x_s[i],
    keeping only the trailing time windows each 128-state block needs
    (|lam|^e below threshold beyond that).  Sorted blocks need ~[8,1,1,1]
    windows of 128 instead of 4x8.
  * Second half: twist e = e^{-ij theta}.Bu decouples the complex recurrence
    into two real per-partition scans s_j = r*s_{j-1} + e_j
    (tensor_tensor_scan), then untwist h = e^{+ij theta}.s.
  * Twist/untwist products done as ONE [P,2048] tensor_tensor per
    (state-block, time-chunk, stage) using multi-dim access patterns
    (table repeated via stride-0 dim), then 2 [P,512] combines.
  * Output GEMMs: 8 PSUM groups (2 time chunks x 4 out blocks) of 12
    accumulated matmuls (C re/im per state block + D).
"""
import numpy as np

import concourse.bass as bass
import concourse.bacc as bacc
import concourse.mybir as mybir
from concourse.tile import TileContext
from concourse.bass_utils import run_bass_kernel_spmd

import cplx_op

BATCH, T, IN, OUT, N = 8, 2048, 512, 512, 512
DS = 1024
P = 128
NB = N // P     # 4 state blocks
IBN = IN // P   # 4 input blocks
OBN = OUT // P  # 4 output blocks
HF = 1024       # first-half length (= DS)
HH = 512        # chunk length / PSUM free-dim
NW = HF // P    # 8 time windows in the first half
SL = 2 * HH + 4  # per-chunk interleaved stream length (seeds + data + pad)

f32 = mybir.dt.float32
f16 = mybir.dt.float16
AOP = mybir.AluOpType

_CACHE = {}


def _build_nc(wins):
    """wins: tuple of per-state-block window counts for the carry GEMM."""
    key = ("nc", wins)
    if key in _CACHE:
        return _CACHE[key]
    nwtot = sum(wins)
    nc = bacc.Bacc()
    xT = nc.dram_tensor("xT", [IN, DS], f16, kind="ExternalInput")
    xh = nc.dram_tensor("xh", [HF, IN], f16, kind="ExternalInput")
    btr = nc.dram_tensor("btr", [IN, N], f16, kind="ExternalInput")
    bti = nc.dram_tensor("bti", [IN, N], f16, kind="ExternalInput")
    vre = nc.dram_tensor("vre", [P, nwtot * P], f16, kind="ExternalInput")
    vim = nc.dram_tensor("vim", [P, nwtot * P], f16, kind="ExternalInput")
    # cs: per state-row, per chunk: [0,0, (cos_j, -sin_j) pairs, 0,0]
    cs = nc.dram_tensor("cs", [N, 2 * SL], f16, kind="ExternalInput")
    bn2 = nc.dram_tensor("bn2", [N, 2 * IN], f16, kind="ExternalInput")
    bn3 = nc.dram_tensor("bn3", [N, 2 * IN], f16, kind="ExternalInput")
    rb = nc.dram_tensor("rb", [N, 1], f32, kind="ExternalInput")
    # rot columns: 0=cos(theta), 1=-sin(theta), 2=sin(theta)
    rot = nc.dram_tensor("rot", [N, 3], f32, kind="ExternalInput")
    ctr = nc.dram_tensor("ctr", [N, OUT], f16, kind="ExternalInput")
    ctin = nc.dram_tensor("ctin", [N, OUT], f16, kind="ExternalInput")
    dtw = nc.dram_tensor("dtw", [IN, OUT], f16, kind="ExternalInput")
    yT = nc.dram_tensor("yT", [OUT, DS], f32, kind="ExternalOutput")

    with TileContext(nc) as tc:
        with (
            tc.tile_pool(name="const", bufs=1) as cp,
            tc.tile_pool(name="ps_in", bufs=4, space="PSUM") as bp,
            tc.tile_pool(name="ps_y", bufs=4, space="PSUM") as yp,
            tc.tile_pool(name="work", bufs=2) as wp,
            tc.tile_pool(name="persist", bufs=1) as hp,
            tc.tile_pool(name="small", bufs=1) as kp,
            tc.tile_pool(name="ysb", bufs=2) as op_,
        ):
            def load_const(dram, rows, dtype, tagp, eng):
                nb_ = rows // P
                cols = dram.shape[1]
                big = cp.tile([P, nb_ * cols], dtype, tag=tagp, name=tagp)
                if nb_ == 1:
                    eng.dma_start(big[:], dram[:])
                else:
                    eng.dma_start(
                        big[:].rearrange("p (b c) -> p b c", b=nb_),
                        dram[:].rearrange("(b p) c -> p b c", p=P))
                return [big[:, i * cols:(i + 1) * cols] for i in range(nb_)]

            # ---- DMA issue order drives availability ----
            xh_t = load_const(xh, HF, f16, "xh", nc.sync)
            vre_t = load_const(vre, P, f16, "vre", nc.scalar)[0]
            vim_t = load_const(vim, P, f16, "vim", nc.scalar)[0]
            btr_t = load_const(btr, IN, f16, "btr", nc.sync)
            xts = load_const(xT, IN, f16, "xt", nc.sync)
            bti_t = load_const(bti, IN, f16, "bti", nc.scalar)
            cs_t = load_const(cs, N, f16, "cs", nc.sync)
            bn2_t = load_const(bn2, N, f16, "bn2", nc.scalar)
            bn3_t = load_const(bn3, N, f16, "bn3", nc.scalar)
            rb_t = load_const(rb, N, f32, "rb", nc.sync)
            rot_t = load_const(rot, N, f32, "rot", nc.sync)
            ctr_t = load_const(ctr, N, f16, "ctr", nc.sync)
            ctin_t = load_const(ctin, N, f16, "ctin", nc.sync)
            dtw_t = load_const(dtw, IN, f16, "dtw", nc.sync)

            # ---- input GEMMs for nb0 first (unblocks the vector pipeline) --
            bus = [None] * NB

            def emit_bu(nb):
                nsl = slice(nb * P, (nb + 1) * P)
                bu = hp.tile([P, 2 * SL], f16, tag=f"bu{nb}", name=f"bu{nb}")
                for ch in range(2):
                    base = ch * SL
                    for pi, w_tiles in enumerate((btr_t, bti_t)):
                        ps = bp.tile([P, HH], f32, tag="psin", name="psin")
                        for ib in range(IBN):
                            nc.tensor.matmul(
                                ps[:],
                                w_tiles[ib][:, nsl],
                                xts[ib][:, ch * HH:(ch + 1) * HH],
                                start=(ib == 0),
                                stop=(ib == IBN - 1),
                            )
                        dst = (bu[:, base + 2 + pi:base + 2 + pi + 2 * HH]
                               .rearrange("p (c two) -> p c two", two=2)
                               [:, :, 0:1])
                        nc.scalar.copy(
                            dst, ps[:].rearrange("p (c o) -> p c o", o=1))
                bus[nb] = bu

            emit_bu(0)

            # ---- carry W-GEMMs (truncated), w2 = [Wre | Wim] per nb ----
            w2s = []
            woff = 0
            for nb in range(NB):
                w_t = wp.tile([P, 2 * IN], f16, tag="w2", name="w2")
                for pi, v_t in enumerate((vre_t, vim_t)):
                    ps = bp.tile([P, IN], f32, tag="psin", name="psin")
                    for wi in range(wins[nb]):
                        sw = NW - wins[nb] + wi  # trailing windows
                        nc.tensor.matmul(
                            ps[:],
                            v_t[:, (woff + wi) * P:(woff + wi + 1) * P],
                            xh_t[sw][:],
                            start=(wi == 0),
                            stop=(wi == wins[nb] - 1),
                        )
                    nc.scalar.copy(w_t[:, pi * IN:(pi + 1) * IN], ps[:])
                woff += wins[nb]
                w2s.append(w_t)

            # ---- carry dots + init rotation per nb ----
            inits = []
            for nb in range(NB):
                dump = wp.tile([P, 2 * IN], f16, tag="dump", name="dump")
                a_re = kp.tile([P, 1], f32, tag=f"are{nb}", name=f"are{nb}")
                a_im = kp.tile([P, 1], f32, tag=f"aim{nb}", name=f"aim{nb}")
                nc.vector.scalar_tensor_tensor(
                    dump[:], bn2_t[nb][:], 1.0, w2s[nb][:], AOP.bypass, AOP.mult,
                    accum_out=a_re[:])
                nc.vector.scalar_tensor_tensor(
                    dump[:], bn3_t[nb][:], 1.0, w2s[nb][:], AOP.bypass, AOP.mult,
                    accum_out=a_im[:])
                i_re = kp.tile([P, 1], f32, tag=f"ire{nb}", name=f"ire{nb}")
                i_im = kp.tile([P, 1], f32, tag=f"iim{nb}", name=f"iim{nb}")
                u_re = kp.tile([P, 1], f32, tag=f"ure{nb}", name=f"ure{nb}")
                u_im = kp.tile([P, 1], f32, tag=f"uim{nb}", name=f"uim{nb}")
                nc.scalar.mul(u_re[:], a_re[:], rot_t[nb][:, 0:1])
                nc.vector.scalar_tensor_tensor(
                    i_re[:], a_im[:], rot_t[nb][:, 1:2], u_re[:],
                    AOP.mult, AOP.add)
                nc.scalar.mul(u_im[:], a_im[:], rot_t[nb][:, 0:1])
                nc.vector.scalar_tensor_tensor(
                    i_im[:], a_re[:], rot_t[nb][:, 2:3], u_im[:],
                    AOP.mult, AOP.add)
                inits.append((i_re, i_im))

            # ---- remaining input GEMMs + chunk-0 seeds ----
            for nb in range(1, NB):
                emit_bu(nb)
            for nb in range(NB):
                i_re, i_im = inits[nb]
                nc.scalar.copy(bus[nb][:, 0:1], i_re[:])
                nc.scalar.copy(bus[nb][:, 1:2], i_im[:])

            # ---- D@x pre-fill of chunk-0 output PSUM groups (PE gap filler) -
            y_ps = {}
            for ob in range(OBN):
                osl = slice(ob * P, (ob + 1) * P)
                ps = yp.tile([P, HH], f32, tag="psy", name="psy")
                y_ps[(0, ob)] = ps
                for ib in range(IBN):
                    nc.tensor.matmul(ps[:], dtw_t[ib][:, osl],
                                     xts[ib][:, 0:HH],
                                     start=(ib == 0), stop=False)

            # ---- fused twist+scan, conj untwist, de-interleave; chunk-major --
            CPLX, CONJ, FUSED = cplx_op.register_all()
            # s streams per (nb, ch): [j,j,j, sre0, sim0, sre1, sim1, ...]
            s_tiles = [hp.tile([P, 2 * SL + 2], f16, tag=f"s{nb}",
                               name=f"s{nb}") for nb in range(NB)]
            hh_tiles = [hp.tile([P, 4 * HH], f16, tag=f"hh{nb}",
                                name=f"hh{nb}") for nb in range(NB)]
            for ch in range(2):
                for nb in range(NB):
                    bu, st, hht = bus[nb], s_tiles[nb], hh_tiles[nb]
                    base = ch * SL
                    if ch == 1:
                        # seed chunk 1 with last (sre, sim) of chunk 0
                        nc.scalar.copy(bu[:, SL:SL + 2],
                                       st[:, 2 * HH + 1:2 * HH + 3])
                    nc.vector._custom_dve(
                        FUSED,
                        out=st[:, base:base + SL],
                        in0=bu[:, base:base + SL],
                        in1=cs_t[nb][:, base:base + SL],
                        s0=rb_t[nb][:, 0:1])
                    # untwist: conj product of s pairs with the same table
                    hhi = wp.tile([P, 2 * HH + 2], f16, tag="hhi", name="hhi")
                    nc.vector._custom_dve(
                        CONJ,
                        out=hhi[:],
                        in0=st[:, base + 3:base + 3 + 2 * HH + 2],
                        in1=cs_t[nb][:, base + 2:base + 2 + 2 * HH + 2])
                    # de-interleave on the vector engine (scalar is busy)
                    for pi in range(2):
                        src = (hhi[:, 1 + pi:1 + pi + 2 * HH]
                               .rearrange("p (c two) -> p c two", two=2)
                               [:, :, 0:1])
                        nc.vector.tensor_copy(
                            hht[:, (2 * ch + pi) * HH:(2 * ch + pi + 1) * HH]
                            .rearrange("p (c o) -> p c o", o=1), src)

                # ---- output GEMMs for this chunk ----
                for ob in range(OBN):
                    osl = slice(ob * P, (ob + 1) * P)
                    if (ch, ob) in y_ps:
                        ps = y_ps[(ch, ob)]
                        ops = []
                    else:
                        ps = yp.tile([P, HH], f32, tag="psy", name="psy")
                        ops = [(dtw_t[ib][:, osl],
                                xts[ib][:, ch * HH:(ch + 1) * HH])
                               for ib in range(IBN)]
                    n_pre = len(ops)
                    for nb in range(NB):
                        ops.append((ctr_t[nb][:, osl],
                                    hh_tiles[nb][:, 2 * ch * HH:(2 * ch + 1) * HH]))
                        ops.append((ctin_t[nb][:, osl],
                                    hh_tiles[nb][:, (2 * ch + 1) * HH:(2 * ch + 2) * HH]))
                    for k, (w, m) in enumerate(ops):
                        nc.tensor.matmul(
                            ps[:], w, m,
                            start=(k == 0 and n_pre > 0),
                            stop=(k == len(ops) - 1))
                    ysb = op_.tile([P, HH], f32, tag="ysb", name="ysb")
                    nc.scalar.copy(ysb[:], ps[:])
                    nc.sync.dma_start(yT[osl, ch * HH:(ch + 1) * HH], ysb[:])

    nc.compile()
    nc.finalize()
    _CACHE[key] = nc
    return nc


def _host_prep(x, nu_log, theta_log, gamma_log, B_re, B_im, C_re, C_im, D):
    f64 = np.float64
    nu = np.asarray(nu_log, f64)
    th = np.asarray(theta_log, f64)
    gl = np.asarray(gamma_log, f64)
    r = np.exp(-np.exp(nu))
    theta = np.exp(th)
    gamma = np.exp(gl)

    # sort states by r descending; permute all per-state data
    perm = np.argsort(-r, kind="stable")
    r = r[perm]
    theta = theta[perm]
    gamma = gamma[perm]
    B_re = np.asarray(B_re, f64)[perm]
    B_im = np.asarray(B_im, f64)[perm]
    C_re = np.asarray(C_re, f64)[:, perm]
    C_im = np.asarray(C_im, f64)[:, perm]

    # per-block trailing-window counts for the carry GEMM
    wins = []
    for nb in range(NB):
        rmax = r[nb * P:(nb + 1) * P].max()
        amp = np.sqrt((1 + rmax) / max(1e-9, 1 - rmax))
        w = 1
        while w < NW and (rmax ** (P * w)) * amp > 2e-4:
            w += 1
        wins.append(int(w))
    wins = tuple(wins)

    gbr = gamma[:, None] * B_re
    gbi = gamma[:, None] * B_im
    shared = {
        "btr": np.ascontiguousarray(gbr.T).astype(np.float16),
        "bti": np.ascontiguousarray(gbi.T).astype(np.float16),
        "ctr": np.ascontiguousarray(C_re.T).astype(np.float16),
        "ctin": np.ascontiguousarray((-C_im).T).astype(np.float16),
        "dtw": np.ascontiguousarray(np.asarray(D, f64).T).astype(np.float16),
        "bn2": np.concatenate([gbr, -gbi], axis=1).astype(np.float16),
        "bn3": np.concatenate([gbi, gbr], axis=1).astype(np.float16),
        "rb": np.ascontiguousarray(r[:, None].astype(np.float32)),
        "rot": np.stack([np.cos(theta), -np.sin(theta), np.sin(theta)],
                        axis=1).astype(np.float32),
    }
    j = np.arange(DS, dtype=f64)
    ang = theta[:, None] * j[None, :]
    cosj = np.cos(ang)
    nsinj = -np.sin(ang)
    # cs rows per chunk: [0, 0, cos_0, -sin_0, cos_1, -sin_1, ..., 0, 0]
    cs = np.zeros((N, 2 * SL), f64)
    for ch in range(2):
        blk = cs[:, ch * SL:(ch + 1) * SL]
        blk[:, 2:2 + 2 * HH:2] = cosj[:, ch * HH:(ch + 1) * HH]
        blk[:, 3:3 + 2 * HH:2] = nsinj[:, ch * HH:(ch + 1) * HH]
    shared["cs"] = cs.astype(np.float16)

    # packed V tables: per nb, `wins[nb]` trailing windows of 128 steps
    vre_bl, vim_bl = [], []
    for nb in range(NB):
        rn = r[nb * P:(nb + 1) * P]
        tn = theta[nb * P:(nb + 1) * P]
        for wi in range(wins[nb]):
            sw = NW - wins[nb] + wi
            s = np.arange(sw * P, (sw + 1) * P, dtype=f64)
            e = (HF - 1) - s
            mag = np.exp(np.log(rn)[None, :] * e[:, None])
            angv = tn[None, :] * e[:, None]
            vre_bl.append(mag * np.cos(angv))   # [s 128, n 128]
            vim_bl.append(mag * np.sin(angv))
    shared["vre"] = np.concatenate(vre_bl, axis=1).astype(np.float16)
    shared["vim"] = np.concatenate(vim_bl, axis=1).astype(np.float16)

    x = np.asarray(x, np.float32)
    in_maps = []
    for b in range(BATCH):
        m = dict(shared)
        m["xT"] = np.ascontiguousarray(x[b, HF:].T).astype(np.float16)
        m["xh"] = np.ascontiguousarray(x[b, :HF]).astype(np.float16)
        in_maps.append(m)
    return in_maps, wins


def _run(in_maps, wins, trace=False):
    nc = _build_nc(wins)
    return run_bass_kernel_spmd(nc, in_maps, core_ids=list(range(BATCH)),
                                trace=trace)


def kernel(**inputs):
    in_maps, wins = _host_prep(**inputs)
    res = _run(in_maps, wins, trace=False)
    y = np.stack([np.ascontiguousarray(res.results[b]["yT"].T)
                  for b in range(BATCH)])
    return y.astype(np.float32)


def kernel_traced(**inputs):
    """Like kernel() but returns (y, exec_time_ns). Used by test.py."""
    in_maps, wins = _host_prep(**inputs)
    res = _run(in_maps, wins, trace=True)
    y = np.stack([np.ascontiguousarray(res.results[b]["yT"].T)
                  for b in range(BATCH)])
    return y.astype(np.float32), res.exec_time_ns
